# revision 25
# baseline (speedup 1.0000x reference)
"""Trainium2 Bass kernel for the DCM sparse-attention problem (v4, bf16).

Same math restructure as the baseline (S-matrix collapse: every softmax
aggregation is a weighted reduction of S[(a,t),(b,v)] = <t,v>/|t||v|),
plus:

- All matmuls in bf16 (1 cyc/row; DMA bytes halved vs f32).
- Video-norm fold AFTER the S matmul (S matmuls start as soon as tiles
  land); tau and r_t fold into that same scalar_tensor_tensor pass, the
  text mask becomes the E-exponential's per-partition scale, and the
  final division absorbs the leftover tau.
- rsqrt = exp(-0.5*ln(x)); Ln and Exp are steered into the one PWP
  table set that holds both, so there is a single hoisted table load
  and no mid-pipeline table switch.
- Warm-up/keepalive junk matmuls hold the PE HAM clock gate open across
  the DMA window and the elementwise mid-section.
- The mid-section is issued in (M-tile, column-half) streams so DVE and
  ACT pipeline instead of serializing on full-width tensors.
- GpSimd does only the mask/const DMAs (SWDGE) and the one rv
  partition-broadcast; its tensor ops are too slow (drains).

Each of the 8 cores handles 8 of the 64 text rows (A-sharded, video
replicated).
"""

import sys

sys.path.insert(0, "/opt/trn_rl_repo")

import ml_dtypes
import numpy as np

import concourse.bass as bass
import concourse.bacc as bacc
import concourse.hw_specs as hw_specs
import concourse.tile as tile
from concourse import mybir
from concourse.bass_utils import run_bass_kernel_spmd

TAU = 100.0
A, T, B, V, D = 64, 32, 64, 12, 512
NCORES = 8
AL = A // NCORES          # a's per core = 8
AT = AL * T               # (a,t) rows per core = 256
BV = B * V                # (b,v) cols = 768
NMT = AT // 128           # M-tiles over (a,t) = 2
NKT = D // 128            # K-tiles over d = 4
APB = 128 // T            # a's per M-tile = 4
F32 = mybir.dt.float32
BF16 = mybir.dt.bfloat16
EXP = mybir.ActivationFunctionType.Exp
LN = mybir.ActivationFunctionType.Ln
MUL = mybir.AluOpType.mult
X = mybir.AxisListType.X
NSL = [(0, 512), (512, 768)]                   # bank-aligned slices of 768
NSL3 = [(0, 512), (512, 1024), (1024, 1536)]   # ... of 1536
HALF = [(0, 384), (384, 768)]                  # group-aligned halves
WSL = [(0, 384), (384, 512), (512, 768)]       # bank-safe W4 chunks
NWARM = 2
JFILL = [3, 3, 2, 2]                           # junk matmuls after k-group k

_JOINT = "natural_log_exp_and_others"
_orig_gat = hw_specs.get_activation_tables


def _gat(arch):
    """Steer Ln and Exp to the one table set containing both, so the
    activation-table pass emits a single load instead of three.  Set ids
    are positional, so entries are filtered in place, never reordered."""
    tables = _orig_gat(arch)
    if _JOINT in tables:
        for name, funcs in tables.items():
            if name != _JOINT:
                funcs.discard(LN)
                funcs.discard(EXP)
    return tables


bacc.get_activation_tables = _gat


def _build_program():
    nc = bacc.Bacc("TRN2", target_bir_lowering=False)

    tT_d = nc.declare_dram_parameter("tT", [128, NKT * AT], BF16, isOutput=False)
    vT_ds = [nc.declare_dram_parameter(f"vT{k}", [128, BV], BF16, isOutput=False)
             for k in range(NKT)]
    mask_d = nc.declare_dram_parameter("mask", [128, NMT], F32, isOutput=False)
    cpack_d = nc.declare_dram_parameter("cpack", [128, NMT * 8 + 1], BF16,
                                        isOutput=False)
    indW_d = nc.declare_dram_parameter("indW", [8, NMT * 128], BF16,
                                       isOutput=False)
    out_d = nc.declare_dram_parameter("out", [AL, B], F32, isOutput=True)

    with tile.TileContext(nc) as tc:
        with (
            tc.tile_pool(name="consts", bufs=1) as consts,
            tc.tile_pool(name="inputs", bufs=1) as inputs,
            tc.tile_pool(name="sq", bufs=1) as sqp,
            tc.tile_pool(name="big", bufs=1) as bigp,
            tc.tile_pool(name="smalls", bufs=1) as smalls,
            tc.tile_pool(name="psA", bufs=2, space="PSUM") as psA,
            tc.tile_pool(name="psB", bufs=1, space="PSUM") as psB,
        ):
            # ---- input DMAs first, split fine-grained across the 3 DGE
            # queues (sync/scalar HWDGE + gpsimd SWDGE) so every k-chunk's
            # pieces land in parallel and in consumption (k) order ----
            vT = inputs.tile([128, NKT * BV], BF16)
            tT = inputs.tile([128, NKT * AT], BF16)
            junk = consts.tile([128, 512], BF16)
            nc.vector.memset(junk, 1.0)

            # coarse per-k transfers: the HW queues fair-share across active
            # transfers, so fewer/bigger transfers finish the head chunks
            # sooner than a fine-grained split
            # scalar carries ONLY tT: it gates the rt chain and every
            # stationary, so it must not fair-share with video transfers
            nc.scalar.dma_start(out=tT[:, :2 * AT], in_=tT_d[:, :2 * AT])
            nc.scalar.dma_start(out=tT[:, 2 * AT:], in_=tT_d[:, 2 * AT:])
            for k in range(2):
                nc.sync.dma_start(out=vT[:, k * BV:(k + 1) * BV],
                                  in_=vT_ds[k][:, :])
            maskt = consts.tile([128, NMT], F32)
            nc.gpsimd.dma_start(out=maskt, in_=mask_d[:, :])
            cpack = consts.tile([128, NMT * 8 + 1], BF16)
            nc.gpsimd.dma_start(out=cpack, in_=cpack_d[:, :])
            nc.gpsimd.dma_start(out=vT[:, 2 * BV:3 * BV], in_=vT_ds[2][:, :])
            nc.gpsimd.dma_start(out=vT[:, 3 * BV:], in_=vT_ds[3][:, :])
            indW = consts.tile([8, NMT * 128], BF16)
            nc.gpsimd.dma_start(out=indW, in_=indW_d[:, :])
            ind36 = cpack[:, :NMT * 8]
            onesc = cpack[:, NMT * 8:]

            # ---- PE warm-up junk feeds the HAM activity monitor; ps_warm
            # shares the tag-s rotation and all junk writes finish before
            # ps_s1 is allocated into the same buffer ----
            ps_warm = psA.tile([128, 512], F32, tag="s")
            for w in range(NWARM):
                nc.tensor.matmul(ps_warm, junk[:, 0:128], junk,
                                 start=True, stop=True)

            sqv = sqp.tile([128, NKT * BV], BF16)
            sqt = sqp.tile([128, NKT * AT], BF16)
            ps_nv = psB.tile([1, BV], F32, tag="v")
            ps_nt = psB.tile([1, AT], F32, tag="j")
            ps_s = [psA.tile([128, BV], F32, tag="s", name=f"ps_s{i}")
                    for i in range(NMT)]
            ident = consts.tile([1, 1], F32)
            nc.vector.memset(ident, 1.0)
            lss = smalls.tile([1, BV + AT], F32)
            rr = smalls.tile([1, BV + AT], F32)
            tau_rt = [smalls.tile([128, 1], F32, name=f"tau_rt{i}")
                      for i in range(NMT)]
            ind36m = [smalls.tile([128, 8], BF16, name=f"ind36m{i}")
                      for i in range(NMT)]

            # ---- text-side norms depend only on tT: do the whole r_t chain
            # up front so it is ready long before the mid-section ----
            for k in range(0, NKT, 2):
                nc.vector.tensor_tensor(
                    sqt[:, k * AT:(k + 2) * AT],
                    tT[:, k * AT:(k + 2) * AT],
                    tT[:, k * AT:(k + 2) * AT], op=MUL)
            for k in range(NKT):
                nc.tensor.matmul(ps_nt, onesc,
                                 sqt[:, k * AT:(k + 1) * AT],
                                 start=(k == 0), stop=(k == NKT - 1))
            nc.scalar.activation(lss[:, BV:], ps_nt, LN)
            nc.scalar.activation(rr[:, BV:], lss[:, BV:], EXP, scale=-0.5)

            # ---- M-tile 0: S matmuls paced by the vT k-chunk DMAs, with
            # video norm matmuls first in each k-group; junk fillers bridge
            # DMA waits so the HAM clock gate stays released ----
            for k in range(NKT):
                nc.vector.tensor_tensor(sqv[:, k * BV:(k + 1) * BV],
                                        vT[:, k * BV:(k + 1) * BV],
                                        vT[:, k * BV:(k + 1) * BV], op=MUL)
                # high priority: the rv chain (nv -> rsqrt -> broadcast) gates
                # the whole mid-section, so nv must preempt the S backlog the
                # moment its sqv chunk is ready
                with tc.high_priority():
                    for lo, hi in NSL:
                        nc.tensor.matmul(ps_nv[:, lo:hi], onesc,
                                         sqv[:, k * BV + lo:k * BV + hi],
                                         start=(k == 0), stop=(k == NKT - 1))
                for lo, hi in NSL:
                    nc.tensor.matmul(
                        ps_s[0][:, lo:hi],
                        tT[:, k * AT:k * AT + 128],
                        vT[:, k * BV + lo:k * BV + hi],
                        start=(k == 0), stop=(k == NKT - 1))
                if k == 1:
                    # r_t transposes slot into a DMA-wait window; the Scalar
                    # rsqrt chain for t is long done by now
                    for i in range(NMT):
                        ps_tr = psB.tile([128, 1], F32, tag="j",
                                         name=f"ps_tr{i}")
                        nc.tensor.transpose(
                            ps_tr, rr[:, BV + 128 * i:BV + 128 * (i + 1)],
                            ident)
                        nc.vector.tensor_scalar_mul(tau_rt[i], ps_tr, TAU)
                        nc.vector.tensor_scalar_mul(ind36m[i],
                                                    ind36[:, 8 * i:8 * (i + 1)],
                                                    maskt[:, i:i + 1])
                # fillers read the just-landed sqv chunk (honest dep) so the
                # Tile scheduler cannot hoist them ahead of this k-group
                for w in range(JFILL[k]):
                    nc.tensor.matmul(
                        ps_warm[:, :256], junk[:, 0:128],
                        sqv[:, k * BV + 256 * w:k * BV + 256 * (w + 1)],
                        start=True, stop=True)

            # ---- video rsqrt + broadcast overlap M-tile 1's S matmuls ----
            rv_bc = bigp.tile([128, BV], F32)
            for lo, hi in HALF:
                nc.scalar.activation(lss[:, lo:hi], ps_nv[:, lo:hi], LN)
                nc.scalar.activation(rr[:, lo:hi], lss[:, lo:hi], EXP,
                                     scale=-0.5)
                nc.gpsimd.partition_broadcast(rv_bc[:, lo:hi], rr[:, lo:hi],
                                              channels=128)

            # ---- M-tile 1: SBUF-resident, runs back-to-back warm ----
            for k in range(NKT):
                for lo, hi in NSL:
                    nc.tensor.matmul(
                        ps_s[1][:, lo:hi],
                        tT[:, k * AT + 128:(k + 1) * AT],
                        vT[:, k * BV + lo:k * BV + hi],
                        start=(k == 0), stop=(k == NKT - 1))

            # ---- mid section in (i, half) streams: sp = tau*r_t*rv*S from
            # PSUM, E = exp(mask*sp), ES = sp*E, then grouped reduces ----
            sp = [bigp.tile([128, BV], BF16, name=f"sp{i}") for i in range(NMT)]
            big = [bigp.tile([128, 2 * BV], BF16, name=f"big{i}")
                   for i in range(NMT)]
            rhs_f = [smalls.tile([128, 128], BF16, name=f"rhs_f{i}")
                     for i in range(NMT)]
            red = [smalls.tile([128, 128], F32, name=f"red{i}")
                   for i in range(NMT)]
            for i in range(NMT):
                for lo, hi in HALF:
                    nc.vector.scalar_tensor_tensor(
                        sp[i][:, lo:hi], ps_s[i][:, lo:hi], tau_rt[i],
                        rv_bc[:, lo:hi], op0=MUL, op1=MUL)
                    nc.scalar.activation(big[i][:, BV + lo:BV + hi],
                                         sp[i][:, lo:hi], EXP,
                                         scale=maskt[:, i:i + 1])
            for i in range(NMT):
                nc.vector.reduce_sum(red[i][:, B:],
                                     big[i][:, BV:].rearrange(
                                         "p (g v) -> p g v", v=V), axis=X)
                for lo, hi in HALF:
                    nc.vector.tensor_tensor(big[i][:, lo:hi], sp[i][:, lo:hi],
                                            big[i][:, BV + lo:BV + hi], op=MUL)
                nc.vector.reduce_sum(red[i][:, :B],
                                     big[i][:, :BV].rearrange(
                                         "p (g v) -> p g v", v=V), axis=X)
            for i in range(NMT):
                rdn = smalls.tile([128, B], F32, name=f"rdn{i}")
                nc.vector.reciprocal_approx_fast(rdn, red[i][:, B:])
                t2v = smalls.tile([128, B], F32, name=f"t2v{i}")
                nc.vector.tensor_tensor(t2v, red[i][:, :B], rdn, op=MUL)
                nc.scalar.activation(rhs_f[i][:, B:], t2v, EXP)

            # ---- PE keepalive while DVE/ACT chew the mid-section (ps_nv is
            # dead once the rsqrt chain has consumed it) ----
            for w in range(3):
                nc.tensor.matmul(ps_nv[:, 0:512], onesc, junk,
                                 start=True, stop=True)

            # ---- v2t: mask-folded indicator matmul over t; rhs is [ES|E] ----
            ps_v = psB.tile([8, 2 * BV], F32, tag="v")
            for i in range(NMT):
                for lo, hi in NSL3:
                    nc.tensor.matmul(ps_v[:, lo:hi], ind36m[i],
                                     big[i][:, lo:hi],
                                     start=(i == 0), stop=(i == NMT - 1))

            # ---- vps2 path at [36, x], half-split so DVE/ACT pipeline ----
            fe4 = bigp.tile([8, BV], BF16)
            d4 = smalls.tile([8, B], F32)
            for lo, hi in HALF:
                rdv = smalls.tile([8, 384], F32, name=f"rdv{lo}")
                nc.vector.reciprocal_approx_fast(rdv, ps_v[:8, BV + lo:BV + hi])
                v2t = smalls.tile([8, 384], F32, name=f"v2t{lo}")
                nc.vector.tensor_tensor(v2t, ps_v[:8, lo:hi], rdv, op=MUL)
                nc.scalar.activation(fe4[:, lo:hi], v2t, EXP)

            # ---- keepalive during the fe4 chain (ps_s slots are dead) ----
            for w in range(2):
                nc.tensor.matmul(ps_s[0][:, 0:512], junk[:, 0:128], junk,
                                 start=True, stop=True)

            # ---- broadcast E4 over t-rows (PE), weight by sp, group-sum ----
            for i in range(NMT):
                ps_w = psA.tile([128, BV], F32, tag="s", name=f"ps_w{i}")
                for lo, hi in WSL:
                    nc.tensor.matmul(ps_w[:, lo:hi],
                                     indW[:, 128 * i:128 * (i + 1)],
                                     fe4[:, lo:hi], start=True, stop=True)
                w4s = sqp.tile([128, BV], BF16, name=f"w4s{i}")
                hun = smalls.tile([128, B], F32, name=f"hun{i}")
                for lo, hi in HALF:
                    nc.vector.tensor_tensor(w4s[:, lo:hi], ps_w[:, lo:hi],
                                            sp[i][:, lo:hi], op=MUL)
                nc.vector.reduce_sum(hun,
                                     w4s.rearrange("p (g v) -> p g v", v=V),
                                     axis=X)
                nc.vector.tensor_tensor(rhs_f[i][:, :B], rhs_f[i][:, B:],
                                        hun, op=MUL)

            ps_o = psB.tile([8, 128], F32, tag="j")
            for i in range(NMT):
                nc.tensor.matmul(ps_o, ind36[:, 8 * i:8 * (i + 1)], rhs_f[i],
                                 start=(i == 0), stop=(i == NMT - 1))
            # d4 is only needed for the final denominator: reduce it late so
            # it does not sit on the DVE queue ahead of the hun reduces
            nc.vector.reduce_sum(d4,
                                 fe4.rearrange("p (g v) -> p g v", v=V),
                                 axis=X)
            d4t = smalls.tile([8, B], F32)
            nc.vector.tensor_scalar_mul(d4t, d4, TAU)
            dd = smalls.tile([8, B], F32)
            nc.vector.tensor_tensor(dd, ps_o[:8, B:], d4t, op=MUL)
            rdd = smalls.tile([8, B], F32)
            nc.vector.reciprocal_approx_fast(rdd, dd)
            outw = smalls.tile([8, B], F32)
            nc.vector.tensor_tensor(outw, ps_o[:8, :B], rdd, op=MUL)
            nc.sync.dma_start(out=out_d[:, :], in_=outw[:, :])

    nc.compile()
    return nc


_NC_CACHE = None


def _get_program():
    global _NC_CACHE
    if _NC_CACHE is None:
        _NC_CACHE = _build_program()
    return _NC_CACHE


def _make_in_maps(text_feat, video_feat, text_mask):
    # vT packed k-major, one DRAM tensor per k-chunk:
    # vT_k[p, c] = video[(b,v)=c, d=128k+p]
    vflat = video_feat.reshape(BV, D).astype(ml_dtypes.bfloat16)
    vT_ks = [np.ascontiguousarray(vflat.T[128 * k:128 * (k + 1), :])
             for k in range(NKT)]
    # ind36 slice i: column 4i + p//T is the block indicator; rows are
    # compact (4 per M-tile, 8 total) so every psum row is live.
    ind36 = np.zeros((128, NMT * 8), np.float32)
    for i in range(NMT):
        for p in range(128):
            ind36[p, 8 * i + 4 * i + p // T] = 1.0
    cpack = np.ones((128, NMT * 8 + 1), ml_dtypes.bfloat16)
    cpack[:, :NMT * 8] = ind36.astype(ml_dtypes.bfloat16)
    # indW slice i: [8, 128] with indW[r, p] = (r == 4i + p//T)
    indW = np.zeros((8, NMT * 128), ml_dtypes.bfloat16)
    for i in range(NMT):
        for p in range(128):
            indW[4 * i + p // T, 128 * i + p] = 1.0
    in_maps = []
    for c in range(NCORES):
        tsl = text_feat[c * AL:(c + 1) * AL].reshape(AT, D) \
            .astype(ml_dtypes.bfloat16)
        tT_b = np.ascontiguousarray(
            tsl.T.reshape(NKT, 128, AT).transpose(1, 0, 2)
            .reshape(128, NKT * AT))
        mask2 = np.ascontiguousarray(
            text_mask[c * AL:(c + 1) * AL].reshape(NMT, 128).T
            .astype(np.float32))
        im = {
            "tT": tT_b,
            "mask": mask2,
            "cpack": cpack,
            "indW": indW,
        }
        for k in range(NKT):
            im[f"vT{k}"] = vT_ks[k]
        in_maps.append(im)
    return in_maps


def kernel(text_feat, video_feat, text_mask, _trace=False):
    text_feat = np.asarray(text_feat, dtype=np.float32)
    video_feat = np.asarray(video_feat, dtype=np.float32)
    text_mask = np.asarray(text_mask)
    nc = _get_program()
    in_maps = _make_in_maps(text_feat, video_feat, text_mask)
    res = run_bass_kernel_spmd(nc, in_maps, core_ids=list(range(NCORES)),
                               trace=_trace)
    out = np.concatenate([res.results[c]["out"] for c in range(NCORES)], axis=0)
    if _trace:
        kernel.last_exec_time_ns = res.exec_time_ns
        kernel.last_results = res
    return out



# revision 39
# speedup vs baseline: 1.1817x; 1.1817x over previous
"""Trainium2 Bass kernel for the DCM sparse-attention problem (v5, bf16).

Same math restructure as the baseline (S-matrix collapse: every softmax
aggregation is a weighted reduction of S[(a,t),(b,v)] = <t,v>/|t||v|),
plus:

- All matmuls in bf16 (1 cyc/row; DMA bytes halved vs f32).
- Video-norm fold AFTER the S matmul; tau and r_t fold into the same
  scalar_tensor_tensor pass, the text mask becomes the E-exponential's
  per-partition scale, and the final division absorbs the leftover tau.
- rsqrt = exp(-0.5*ln(x)); Ln and Exp share one PWP table set.
- M-tile-OUTER loop: M-tile 0's S matmuls pace with the vT k-chunk DMA
  arrivals; M-tile 1 runs back-to-back from SBUF afterwards, overlapped
  with the video rsqrt chain and the start of the mid-section.
- The text-side norm chain (ones-matmuls over sqt, rsqrt, r_t
  transposes) runs up front: it depends only on tT, which gets the
  scalar DGE queue to itself so it lands first.
- nv norm matmuls are high-priority: their k3 retire gates the whole
  mid-section (rv rsqrt -> partition broadcast -> sp).
- Warmup + per-k-group junk filler matmuls (with honest deps on the
  just-landed sqv chunk so the scheduler cannot hoist them) keep the PE
  HAM activity window busy through the DMA phase; no filler after k3 so
  nv k3 retires immediately.
- Mid-section: ES products issue before the [ES|E] merged grouped
  reduces (one reduce per M-tile); the t2v/rhs block runs after the fe4
  chain since ps_o consumes it much later.
- w4s multiplies split across engines: one half direct-from-PSUM on
  DVE, the other drained to bf16 by a Scalar copy so its DVE multiply
  runs 2x packed; the d4 reduce is deferred to the very end.

Each of the 8 cores handles 8 of the 64 text rows (A-sharded, video
replicated, one DRAM tensor per vT k-chunk).
"""

import sys

sys.path.insert(0, "/opt/trn_rl_repo")

import ml_dtypes
import numpy as np

import concourse.bass as bass
import concourse.bacc as bacc
import concourse.hw_specs as hw_specs
import concourse.tile as tile
from concourse import mybir
from concourse.bass_utils import run_bass_kernel_spmd

TAU = 100.0
A, T, B, V, D = 64, 32, 64, 12, 512
NCORES = 8
AL = A // NCORES          # a's per core = 8
AT = AL * T               # (a,t) rows per core = 256
BV = B * V                # (b,v) cols = 768
NMT = AT // 128           # M-tiles over (a,t) = 2
NKT = D // 128            # K-tiles over d = 4
APB = 128 // T            # a's per M-tile = 4
F32 = mybir.dt.float32
BF16 = mybir.dt.bfloat16
EXP = mybir.ActivationFunctionType.Exp
LN = mybir.ActivationFunctionType.Ln
MUL = mybir.AluOpType.mult
X = mybir.AxisListType.X
NSL = [(0, 512), (512, 768)]                   # bank-aligned slices of 768
NSL3 = [(0, 512), (512, 1024), (1024, 1536)]   # ... of 1536
HALF = [(0, 384), (384, 768)]                  # group-aligned halves
WSL = [(0, 384), (384, 512), (512, 768)]       # bank-safe W4 chunks
NWARM = 5
JFILL = [4, 3, 2, 0]                           # junk matmuls after k-group k
                                               # (none after k3: nv k3 gates
                                               # the whole mid-section)

_JOINT = "natural_log_exp_and_others"
_orig_gat = hw_specs.get_activation_tables


def _gat(arch):
    """Steer Ln and Exp to the one table set containing both, so the
    activation-table pass emits a single load instead of three.  Set ids
    are positional, so entries are filtered in place, never reordered."""
    tables = _orig_gat(arch)
    if _JOINT in tables:
        for name, funcs in tables.items():
            if name != _JOINT:
                funcs.discard(LN)
                funcs.discard(EXP)
    return tables


bacc.get_activation_tables = _gat


def _build_program():
    nc = bacc.Bacc("TRN2", target_bir_lowering=False)

    tT_d = nc.declare_dram_parameter("tT", [128, NKT * AT], BF16, isOutput=False)
    vT_ds = [nc.declare_dram_parameter(f"vT{k}", [128, BV], BF16, isOutput=False)
             for k in range(NKT)]
    mask_d = nc.declare_dram_parameter("mask", [128, NMT], F32, isOutput=False)
    cpack_d = nc.declare_dram_parameter("cpack", [128, NMT * 8 + 1], BF16,
                                        isOutput=False)
    indW_d = nc.declare_dram_parameter("indW", [8, NMT * 128], BF16,
                                       isOutput=False)
    out_d = nc.declare_dram_parameter("out", [AL, B], F32, isOutput=True)

    with tile.TileContext(nc) as tc:
        with (
            tc.tile_pool(name="consts", bufs=1) as consts,
            tc.tile_pool(name="inputs", bufs=1) as inputs,
            tc.tile_pool(name="sq", bufs=1) as sqp,
            tc.tile_pool(name="big", bufs=1) as bigp,
            tc.tile_pool(name="smalls", bufs=1) as smalls,
            tc.tile_pool(name="psA", bufs=2, space="PSUM") as psA,
            tc.tile_pool(name="psB", bufs=1, space="PSUM") as psB,
        ):
            # ---- input DMAs first, split fine-grained across the 3 DGE
            # queues (sync/scalar HWDGE + gpsimd SWDGE) so every k-chunk's
            # pieces land in parallel and in consumption (k) order ----
            vT = inputs.tile([128, NKT * BV], BF16)
            tT = inputs.tile([128, NKT * AT], BF16)
            junk = consts.tile([128, 512], BF16)
            nc.vector.memset(junk, 1.0)

            # coarse per-k transfers: the HW queues fair-share across active
            # transfers, so fewer/bigger transfers finish the head chunks
            # sooner than a fine-grained split
            # scalar carries ONLY tT: it gates the rt chain and every
            # stationary, so it must not fair-share with video transfers
            nc.scalar.dma_start(out=tT[:, :2 * AT], in_=tT_d[:, :2 * AT])
            nc.scalar.dma_start(out=tT[:, 2 * AT:], in_=tT_d[:, 2 * AT:])
            for k in range(2):
                nc.sync.dma_start(out=vT[:, k * BV:(k + 1) * BV],
                                  in_=vT_ds[k][:, :])
            maskt = consts.tile([128, NMT], F32)
            nc.gpsimd.dma_start(out=maskt, in_=mask_d[:, :])
            cpack = consts.tile([128, NMT * 8 + 1], BF16)
            nc.gpsimd.dma_start(out=cpack, in_=cpack_d[:, :])
            nc.gpsimd.dma_start(out=vT[:, 2 * BV:3 * BV], in_=vT_ds[2][:, :])
            nc.gpsimd.dma_start(out=vT[:, 3 * BV:], in_=vT_ds[3][:, :])
            indW = consts.tile([8, NMT * 128], BF16)
            nc.gpsimd.dma_start(out=indW, in_=indW_d[:, :])
            ind36 = cpack[:, :NMT * 8]
            onesc = cpack[:, NMT * 8:]

            # ---- PE warm-up junk feeds the HAM activity monitor; ps_warm
            # shares the tag-s rotation and all junk writes finish before
            # ps_s1 is allocated into the same buffer ----
            ps_warm = psA.tile([128, 512], F32, tag="s")
            for w in range(NWARM):
                nc.tensor.matmul(ps_warm, junk[:, 0:128], junk,
                                 start=True, stop=True)

            sqv = sqp.tile([128, NKT * BV], BF16)
            sqt = sqp.tile([128, NKT * AT], BF16)
            ps_nv = psB.tile([1, BV], F32, tag="v")
            ps_nt = psB.tile([1, AT], F32, tag="j")
            ps_s = [psA.tile([128, BV], F32, tag="s", name=f"ps_s{i}")
                    for i in range(NMT)]
            ident = consts.tile([1, 1], F32)
            nc.vector.memset(ident, 1.0)
            lss = smalls.tile([1, BV + AT], F32)
            rr = smalls.tile([1, BV + AT], F32)
            tau_rt = [smalls.tile([128, 1], F32, name=f"tau_rt{i}")
                      for i in range(NMT)]
            ind36m = [smalls.tile([128, 8], BF16, name=f"ind36m{i}")
                      for i in range(NMT)]

            # ---- text-side norms depend only on tT: do the whole r_t chain
            # up front so it is ready long before the mid-section ----
            for k in range(0, NKT, 2):
                nc.vector.tensor_tensor(
                    sqt[:, k * AT:(k + 2) * AT],
                    tT[:, k * AT:(k + 2) * AT],
                    tT[:, k * AT:(k + 2) * AT], op=MUL)
            for k in range(NKT):
                nc.tensor.matmul(ps_nt, onesc,
                                 sqt[:, k * AT:(k + 1) * AT],
                                 start=(k == 0), stop=(k == NKT - 1))
            nc.scalar.activation(lss[:, BV:], ps_nt, LN)
            nc.scalar.activation(rr[:, BV:], lss[:, BV:], EXP, scale=-0.5)

            # ---- M-tile 0: S matmuls paced by the vT k-chunk DMAs, with
            # video norm matmuls first in each k-group; junk fillers bridge
            # DMA waits so the HAM clock gate stays released ----
            for k in range(NKT):
                nc.vector.tensor_tensor(sqv[:, k * BV:(k + 1) * BV],
                                        vT[:, k * BV:(k + 1) * BV],
                                        vT[:, k * BV:(k + 1) * BV], op=MUL)
                # high priority: the rv chain (nv -> rsqrt -> broadcast) gates
                # the whole mid-section, so nv must preempt the S backlog the
                # moment its sqv chunk is ready
                with tc.high_priority():
                    for lo, hi in NSL:
                        nc.tensor.matmul(ps_nv[:, lo:hi], onesc,
                                         sqv[:, k * BV + lo:k * BV + hi],
                                         start=(k == 0), stop=(k == NKT - 1))
                for lo, hi in NSL:
                    nc.tensor.matmul(
                        ps_s[0][:, lo:hi],
                        tT[:, k * AT:k * AT + 128],
                        vT[:, k * BV + lo:k * BV + hi],
                        start=(k == 0), stop=(k == NKT - 1))
                if k == 1:
                    # r_t transposes slot into a DMA-wait window; the Scalar
                    # rsqrt chain for t is long done by now
                    for i in range(NMT):
                        ps_tr = psB.tile([128, 1], F32, tag="j",
                                         name=f"ps_tr{i}")
                        nc.tensor.transpose(
                            ps_tr, rr[:, BV + 128 * i:BV + 128 * (i + 1)],
                            ident)
                        nc.vector.tensor_scalar_mul(tau_rt[i], ps_tr, TAU)
                        nc.vector.tensor_scalar_mul(ind36m[i],
                                                    ind36[:, 8 * i:8 * (i + 1)],
                                                    maskt[:, i:i + 1])
                # fillers read the just-landed sqv chunk (honest dep) so the
                # Tile scheduler cannot hoist them ahead of this k-group
                for w in range(JFILL[k]):
                    o = 256 * (w % 3)
                    nc.tensor.matmul(
                        ps_warm[:, :256], junk[:, 0:128],
                        sqv[:, k * BV + o:k * BV + o + 256],
                        start=True, stop=True)

            # ---- video rsqrt + broadcast overlap M-tile 1's S matmuls ----
            rv_bc = bigp.tile([128, BV], F32)
            for lo, hi in HALF:
                with tc.high_priority():
                    nc.scalar.activation(lss[:, lo:hi], ps_nv[:, lo:hi], LN)
                    nc.scalar.activation(rr[:, lo:hi], lss[:, lo:hi], EXP,
                                         scale=-0.5)
                    nc.gpsimd.partition_broadcast(rv_bc[:, lo:hi],
                                                  rr[:, lo:hi], channels=128)

            # ---- M-tile 1: SBUF-resident, runs back-to-back warm ----
            for k in range(NKT):
                for lo, hi in NSL:
                    nc.tensor.matmul(
                        ps_s[1][:, lo:hi],
                        tT[:, k * AT + 128:(k + 1) * AT],
                        vT[:, k * BV + lo:k * BV + hi],
                        start=(k == 0), stop=(k == NKT - 1))

            # ---- mid section in (i, half) streams: sp = tau*r_t*rv*S from
            # PSUM, E = exp(mask*sp), ES = sp*E, then grouped reduces ----
            sp = [bigp.tile([128, BV], BF16, name=f"sp{i}") for i in range(NMT)]
            big = [bigp.tile([128, 2 * BV], BF16, name=f"big{i}")
                   for i in range(NMT)]
            rhs_f = [smalls.tile([128, 128], BF16, name=f"rhs_f{i}")
                     for i in range(NMT)]
            red = [smalls.tile([128, 128], F32, name=f"red{i}")
                   for i in range(NMT)]
            for i in range(NMT):
                for lo, hi in HALF:
                    nc.vector.scalar_tensor_tensor(
                        sp[i][:, lo:hi], ps_s[i][:, lo:hi], tau_rt[i],
                        rv_bc[:, lo:hi], op0=MUL, op1=MUL)
                    nc.scalar.activation(big[i][:, BV + lo:BV + hi],
                                         sp[i][:, lo:hi], EXP,
                                         scale=maskt[:, i:i + 1])
            # ES multiplies first: they gate the v2t indicator matmuls (the
            # longest downstream chain); the reduces then fill DVE time while
            # PE/Scalar chew on the v2t->fe4 path
            for i in range(NMT):
                for lo, hi in HALF:
                    nc.vector.tensor_tensor(big[i][:, lo:hi], sp[i][:, lo:hi],
                                            big[i][:, BV + lo:BV + hi], op=MUL)
            # ---- PE keepalive while DVE/ACT chew the mid-section (ps_nv is
            # dead once the rsqrt chain has consumed it) ----
            for w in range(3):
                nc.tensor.matmul(ps_nv[:, 0:512], onesc, junk,
                                 start=True, stop=True)

            # ---- v2t: mask-folded indicator matmul over t; rhs is [ES|E] ----
            ps_v = psB.tile([8, 2 * BV], F32, tag="v")
            for i in range(NMT):
                for lo, hi in NSL3:
                    nc.tensor.matmul(ps_v[:, lo:hi], ind36m[i],
                                     big[i][:, lo:hi],
                                     start=(i == 0), stop=(i == NMT - 1))

            for i in range(NMT):
                # single grouped reduce over [ES | E]: columns 0:B are the
                # ES-sums, B:2B the E-sums; fills DVE time while PE runs the
                # ps_v matmuls
                nc.vector.reduce_sum(red[i],
                                     big[i].rearrange(
                                         "p (g v) -> p g v", v=V), axis=X)

            # ---- vps2 path at [36, x], half-split so DVE/ACT pipeline ----
            fe4 = bigp.tile([8, BV], BF16)
            d4 = smalls.tile([8, B], F32)
            v2ts = []
            for lo, hi in HALF:
                rdv = smalls.tile([8, 384], F32, name=f"rdv{lo}")
                nc.vector.reciprocal_approx_fast(rdv, ps_v[:8, BV + lo:BV + hi])
                v2t = smalls.tile([8, 384], F32, name=f"v2t{lo}")
                nc.vector.tensor_tensor(v2t, ps_v[:8, lo:hi], rdv, op=MUL)
                nc.scalar.activation(fe4[:, lo:hi], v2t, EXP)
                v2ts.append(v2t)



            # t2v/rhs: needed only by the final ps_o matmul, so it runs after
            # the fe4 chain on the DVE queue
            for i in range(NMT):
                rdn = smalls.tile([128, B], F32, name=f"rdn{i}")
                nc.vector.reciprocal_approx_fast(rdn, red[i][:, B:])
                t2v = smalls.tile([128, B], F32, name=f"t2v{i}")
                nc.vector.tensor_tensor(t2v, red[i][:, :B], rdn, op=MUL)
                nc.scalar.activation(rhs_f[i][:, B:], t2v, EXP)

            # ---- broadcast E4 over t-rows (PE), weight by sp, group-sum ----
            for i in range(NMT):
                ps_w = psA.tile([128, BV], F32, tag="s", name=f"ps_w{i}")
                for lo, hi in WSL:
                    nc.tensor.matmul(ps_w[:, lo:hi],
                                     indW[:, 128 * i:128 * (i + 1)],
                                     fe4[:, lo:hi], start=True, stop=True)
                w4s = sqp.tile([128, BV], BF16, name=f"w4s{i}")
                w4c = sqp.tile([128, BV], BF16, name=f"w4c{i}")
                hun = smalls.tile([128, B], F32, name=f"hun{i}")
                # split the ps_w*sp multiply across engines: half 0 direct
                # from PSUM on DVE, half 1 drained to bf16 by Scalar first
                # so its DVE multiply runs in 2x packed mode
                (l0, h0), (l1, h1) = HALF
                nc.scalar.copy(w4c[:, l1:h1], ps_w[:, l1:h1])
                nc.vector.tensor_tensor(w4s[:, l0:h0], ps_w[:, l0:h0],
                                        sp[i][:, l0:h0], op=MUL)
                nc.vector.tensor_tensor(w4s[:, l1:h1], w4c[:, l1:h1],
                                        sp[i][:, l1:h1], op=MUL)
                nc.vector.reduce_sum(hun,
                                     w4s.rearrange("p (g v) -> p g v", v=V),
                                     axis=X)
                nc.vector.tensor_tensor(rhs_f[i][:, :B], rhs_f[i][:, B:],
                                        hun, op=MUL)

            ps_o = psB.tile([8, 128], F32, tag="j")
            for i in range(NMT):
                nc.tensor.matmul(ps_o, ind36[:, 8 * i:8 * (i + 1)], rhs_f[i],
                                 start=(i == 0), stop=(i == NMT - 1))
            # d4 is only needed for the final denominator: reduce it late so
            # it does not sit on the DVE queue ahead of the hun reduces
            nc.vector.reduce_sum(d4,
                                 fe4.rearrange("p (g v) -> p g v", v=V),
                                 axis=X)
            d4t = smalls.tile([8, B], F32)
            nc.vector.tensor_scalar_mul(d4t, d4, TAU)
            dd = smalls.tile([8, B], F32)
            nc.vector.tensor_tensor(dd, ps_o[:8, B:], d4t, op=MUL)
            rdd = smalls.tile([8, B], F32)
            nc.vector.reciprocal_approx_fast(rdd, dd)
            outw = smalls.tile([8, B], F32)
            nc.vector.tensor_tensor(outw, ps_o[:8, :B], rdd, op=MUL)
            nc.sync.dma_start(out=out_d[:, :], in_=outw[:, :])

    nc.compile()
    return nc


_NC_CACHE = None


def _get_program():
    global _NC_CACHE
    if _NC_CACHE is None:
        _NC_CACHE = _build_program()
    return _NC_CACHE


def _make_in_maps(text_feat, video_feat, text_mask):
    # vT packed k-major, one DRAM tensor per k-chunk:
    # vT_k[p, c] = video[(b,v)=c, d=128k+p]
    vflat = video_feat.reshape(BV, D).astype(ml_dtypes.bfloat16)
    vT_ks = [np.ascontiguousarray(vflat.T[128 * k:128 * (k + 1), :])
             for k in range(NKT)]
    # ind36 slice i: column 4i + p//T is the block indicator; rows are
    # compact (4 per M-tile, 8 total) so every psum row is live.
    ind36 = np.zeros((128, NMT * 8), np.float32)
    for i in range(NMT):
        for p in range(128):
            ind36[p, 8 * i + 4 * i + p // T] = 1.0
    cpack = np.ones((128, NMT * 8 + 1), ml_dtypes.bfloat16)
    cpack[:, :NMT * 8] = ind36.astype(ml_dtypes.bfloat16)
    # indW slice i: [8, 128] with indW[r, p] = (r == 4i + p//T)
    indW = np.zeros((8, NMT * 128), ml_dtypes.bfloat16)
    for i in range(NMT):
        for p in range(128):
            indW[4 * i + p // T, 128 * i + p] = 1.0
    in_maps = []
    for c in range(NCORES):
        tsl = text_feat[c * AL:(c + 1) * AL].reshape(AT, D) \
            .astype(ml_dtypes.bfloat16)
        tT_b = np.ascontiguousarray(
            tsl.T.reshape(NKT, 128, AT).transpose(1, 0, 2)
            .reshape(128, NKT * AT))
        mask2 = np.ascontiguousarray(
            text_mask[c * AL:(c + 1) * AL].reshape(NMT, 128).T
            .astype(np.float32))
        im = {
            "tT": tT_b,
            "mask": mask2,
            "cpack": cpack,
            "indW": indW,
        }
        for k in range(NKT):
            im[f"vT{k}"] = vT_ks[k]
        in_maps.append(im)
    return in_maps


def kernel(text_feat, video_feat, text_mask, _trace=False):
    text_feat = np.asarray(text_feat, dtype=np.float32)
    video_feat = np.asarray(video_feat, dtype=np.float32)
    text_mask = np.asarray(text_mask)
    nc = _get_program()
    in_maps = _make_in_maps(text_feat, video_feat, text_mask)
    res = run_bass_kernel_spmd(nc, in_maps, core_ids=list(range(NCORES)),
                               trace=_trace)
    out = np.concatenate([res.results[c]["out"] for c in range(NCORES)], axis=0)
    if _trace:
        kernel.last_exec_time_ns = res.exec_time_ns
        kernel.last_results = res
    return out



# revision 54
# speedup vs baseline: 1.1991x; 1.0147x over previous
"""Trainium2 Bass kernel for the DCM sparse-attention problem (v5, bf16).

Same math restructure as the baseline (S-matrix collapse: every softmax
aggregation is a weighted reduction of S[(a,t),(b,v)] = <t,v>/|t||v|),
plus:

- All matmuls in bf16 (1 cyc/row; DMA bytes halved vs f32).
- Video-norm fold AFTER the S matmul; tau and r_t fold into the same
  scalar_tensor_tensor pass, the text mask becomes the E-exponential's
  per-partition scale, and the final division absorbs the leftover tau.
- rsqrt = exp(-0.5*ln(x)); Ln and Exp share one PWP table set.
- M-tile-OUTER loop: M-tile 0's S matmuls pace with the vT k-chunk DMA
  arrivals; M-tile 1 runs back-to-back from SBUF afterwards, overlapped
  with the video rsqrt chain and the start of the mid-section.
- The text-side norm chain (ones-matmuls over sqt, rsqrt, r_t
  transposes) runs up front: it depends only on tT, which gets the
  scalar DGE queue to itself so it lands first.
- nv norm matmuls are high-priority: their k3 retire gates the whole
  mid-section (rv rsqrt -> partition broadcast -> sp).
- Warmup + per-k-group junk filler matmuls (with honest deps on the
  just-landed sqv chunk so the scheduler cannot hoist them) keep the PE
  HAM activity window busy through the DMA phase; no filler after k3 so
  nv k3 retires immediately.
- Mid-section: ES products issue before the [ES|E] merged grouped
  reduces (one reduce per M-tile); the t2v/rhs block runs after the fe4
  chain since ps_o consumes it much later.
- w4s multiplies split across engines: one half direct-from-PSUM on
  DVE, the other drained to bf16 by a Scalar copy so its DVE multiply
  runs 2x packed; the d4 reduce is deferred to the very end.

Each of the 8 cores handles 8 of the 64 text rows (A-sharded, video
replicated, one DRAM tensor per vT k-chunk).
"""

import sys

sys.path.insert(0, "/opt/trn_rl_repo")

import ml_dtypes
import numpy as np

import concourse.bass as bass
import concourse.bacc as bacc
import concourse.hw_specs as hw_specs
import concourse.tile as tile
from concourse import mybir
from concourse.bass_utils import run_bass_kernel_spmd

TAU = 100.0
A, T, B, V, D = 64, 32, 64, 12, 512
NCORES = 8
AL = A // NCORES          # a's per core = 8
AT = AL * T               # (a,t) rows per core = 256
BV = B * V                # (b,v) cols = 768
NMT = AT // 128           # M-tiles over (a,t) = 2
NKT = D // 128            # K-tiles over d = 4
APB = 128 // T            # a's per M-tile = 4
F32 = mybir.dt.float32
BF16 = mybir.dt.bfloat16
EXP = mybir.ActivationFunctionType.Exp
LN = mybir.ActivationFunctionType.Ln
MUL = mybir.AluOpType.mult
X = mybir.AxisListType.X
NSL = [(0, 512), (512, 768)]                   # bank-aligned slices of 768
NSL3 = [(0, 512), (512, 1024), (1024, 1536)]   # ... of 1536
HALF = [(0, 384), (384, 768)]                  # group-aligned halves
WSL = [(0, 384), (384, 512), (512, 768)]       # bank-safe W4 chunks
NWARM = 5
JFILL = [4, 3, 2, 0]                           # junk matmuls after k-group k
                                               # (none after k3: nv k3 gates
                                               # the whole mid-section)

_JOINT = "natural_log_exp_and_others"
_orig_gat = hw_specs.get_activation_tables


def _gat(arch):
    """Steer Ln and Exp to the one table set containing both, so the
    activation-table pass emits a single load instead of three.  Set ids
    are positional, so entries are filtered in place, never reordered."""
    tables = _orig_gat(arch)
    if _JOINT in tables:
        for name, funcs in tables.items():
            if name != _JOINT:
                funcs.discard(LN)
                funcs.discard(EXP)
    return tables


bacc.get_activation_tables = _gat


def _build_program():
    nc = bacc.Bacc("TRN2", target_bir_lowering=False)

    tT_d = nc.declare_dram_parameter("tT", [128, NKT * AT], BF16, isOutput=False)
    vT_ds = [nc.declare_dram_parameter(f"vT{k}", [128, BV], BF16, isOutput=False)
             for k in range(NKT)]
    mask_d = nc.declare_dram_parameter("mask", [128, NMT], F32, isOutput=False)
    cpack_d = nc.declare_dram_parameter("cpack", [128, NMT * 8 + 1], BF16,
                                        isOutput=False)
    indW_d = nc.declare_dram_parameter("indW", [8, NMT * 128], BF16,
                                       isOutput=False)
    out_d = nc.declare_dram_parameter("out", [AL, B], F32, isOutput=True)

    with tile.TileContext(nc) as tc:
        with (
            tc.tile_pool(name="consts", bufs=1) as consts,
            tc.tile_pool(name="inputs", bufs=1) as inputs,
            tc.tile_pool(name="sq", bufs=1) as sqp,
            tc.tile_pool(name="big", bufs=1) as bigp,
            tc.tile_pool(name="smalls", bufs=1) as smalls,
            tc.tile_pool(name="psA", bufs=2, space="PSUM") as psA,
            tc.tile_pool(name="psB", bufs=1, space="PSUM") as psB,
        ):
            # ---- input DMAs first, split fine-grained across the 3 DGE
            # queues (sync/scalar HWDGE + gpsimd SWDGE) so every k-chunk's
            # pieces land in parallel and in consumption (k) order ----
            vT = inputs.tile([128, NKT * BV], BF16)
            tT = inputs.tile([128, NKT * AT], BF16)
            junk = consts.tile([128, 512], BF16)
            nc.vector.memset(junk, 1.0)

            # coarse per-k transfers: the HW queues fair-share across active
            # transfers, so fewer/bigger transfers finish the head chunks
            # sooner than a fine-grained split
            # scalar carries ONLY tT: it gates the rt chain and every
            # stationary, so it must not fair-share with video transfers
            nc.scalar.dma_start(out=tT[:, :2 * AT], in_=tT_d[:, :2 * AT])
            nc.scalar.dma_start(out=tT[:, 2 * AT:], in_=tT_d[:, 2 * AT:])
            for k in range(2):
                nc.sync.dma_start(out=vT[:, k * BV:(k + 1) * BV],
                                  in_=vT_ds[k][:, :])
            maskt = consts.tile([128, NMT], F32)
            nc.gpsimd.dma_start(out=maskt, in_=mask_d[:, :])
            cpack = consts.tile([128, NMT * 8 + 1], BF16)
            nc.gpsimd.dma_start(out=cpack, in_=cpack_d[:, :])
            nc.gpsimd.dma_start(out=vT[:, 2 * BV:3 * BV], in_=vT_ds[2][:, :])
            nc.gpsimd.dma_start(out=vT[:, 3 * BV:], in_=vT_ds[3][:, :])
            indW = consts.tile([8, NMT * 128], BF16)
            nc.gpsimd.dma_start(out=indW, in_=indW_d[:, :])
            ind36 = cpack[:, :NMT * 8]
            onesc = cpack[:, NMT * 8:]

            # ---- PE warm-up junk feeds the HAM activity monitor; ps_warm
            # shares the tag-s rotation and all junk writes finish before
            # ps_s1 is allocated into the same buffer ----
            ps_warm = psA.tile([128, 512], F32, tag="s")
            for w in range(NWARM):
                nc.tensor.matmul(ps_warm, junk[:, 0:128], junk,
                                 start=True, stop=True)

            sqv = sqp.tile([128, NKT * BV], BF16)
            sqt = sqp.tile([128, NKT * AT], BF16)
            ps_nv = psB.tile([1, BV], F32, tag="v")
            ps_nt = psB.tile([1, AT], F32, tag="j")
            ps_s = [psA.tile([128, BV], F32, tag="s", name=f"ps_s{i}")
                    for i in range(NMT)]
            ident = consts.tile([1, 1], F32)
            nc.vector.memset(ident, 1.0)
            lss = smalls.tile([1, BV + AT], F32)
            rr = smalls.tile([1, BV + AT], F32)
            tau_rt = [smalls.tile([128, 1], F32, name=f"tau_rt{i}")
                      for i in range(NMT)]
            ind36m = [smalls.tile([128, 8], BF16, name=f"ind36m{i}")
                      for i in range(NMT)]

            # ---- text-side norms depend only on tT: do the whole r_t chain
            # up front so it is ready long before the mid-section ----
            for k in range(0, NKT, 2):
                nc.vector.tensor_tensor(
                    sqt[:, k * AT:(k + 2) * AT],
                    tT[:, k * AT:(k + 2) * AT],
                    tT[:, k * AT:(k + 2) * AT], op=MUL)
            for k in range(NKT):
                nc.tensor.matmul(ps_nt, onesc,
                                 sqt[:, k * AT:(k + 1) * AT],
                                 start=(k == 0), stop=(k == NKT - 1))
            nc.scalar.activation(lss[:, BV:], ps_nt, LN)
            nc.scalar.activation(rr[:, BV:], lss[:, BV:], EXP, scale=-0.5)

            # warm up the gpsimd partition_broadcast ext-isa lib in the DMA
            # shadow: the first call to a freshly-loaded Q7 kernel pays a
            # ~6us hidden IRAM load, which otherwise lands on the rv chain;
            # the sqt dep pins it after the SWDGE descriptor generation
            bc_warm = smalls.tile([128, 8], BF16)
            nc.gpsimd.partition_broadcast(bc_warm, sqt[0:1, 0:8],
                                          channels=128)

            # ---- M-tile 0: S matmuls paced by the vT k-chunk DMAs, with
            # video norm matmuls first in each k-group; junk fillers bridge
            # DMA waits so the HAM clock gate stays released ----
            for k in range(NKT):
                nc.vector.tensor_tensor(sqv[:, k * BV:(k + 1) * BV],
                                        vT[:, k * BV:(k + 1) * BV],
                                        vT[:, k * BV:(k + 1) * BV], op=MUL)
                # high priority: the rv chain (nv -> rsqrt -> broadcast) gates
                # the whole mid-section, so nv must preempt the S backlog the
                # moment its sqv chunk is ready
                with tc.high_priority():
                    for lo, hi in NSL:
                        nc.tensor.matmul(ps_nv[:, lo:hi], onesc,
                                         sqv[:, k * BV + lo:k * BV + hi],
                                         start=(k == 0), stop=(k == NKT - 1))
                for lo, hi in NSL:
                    nc.tensor.matmul(
                        ps_s[0][:, lo:hi],
                        tT[:, k * AT:k * AT + 128],
                        vT[:, k * BV + lo:k * BV + hi],
                        start=(k == 0), stop=(k == NKT - 1))
                if k == 1:
                    # r_t transposes slot into a DMA-wait window; the Scalar
                    # rsqrt chain for t is long done by now
                    for i in range(NMT):
                        ps_tr = psB.tile([128, 1], F32, tag="j",
                                         name=f"ps_tr{i}")
                        nc.tensor.transpose(
                            ps_tr, rr[:, BV + 128 * i:BV + 128 * (i + 1)],
                            ident)
                        nc.vector.tensor_scalar_mul(tau_rt[i], ps_tr, TAU)
                        nc.vector.tensor_scalar_mul(ind36m[i],
                                                    ind36[:, 8 * i:8 * (i + 1)],
                                                    maskt[:, i:i + 1])
                # fillers read the just-landed sqv chunk (honest dep) so the
                # Tile scheduler cannot hoist them ahead of this k-group
                for w in range(JFILL[k]):
                    o = 256 * (w % 3)
                    nc.tensor.matmul(
                        ps_warm[:, :256], junk[:, 0:128],
                        sqv[:, k * BV + o:k * BV + o + 256],
                        start=True, stop=True)

            # ---- video rsqrt + broadcast overlap M-tile 1's S matmuls ----
            rv_bc = bigp.tile([128, BV], F32)
            for lo, hi in HALF:
                with tc.high_priority():
                    nc.scalar.activation(lss[:, lo:hi], ps_nv[:, lo:hi], LN)
                    nc.scalar.activation(rr[:, lo:hi], lss[:, lo:hi], EXP,
                                         scale=-0.5)
                    nc.gpsimd.partition_broadcast(rv_bc[:, lo:hi],
                                                  rr[:, lo:hi], channels=128)

            # ---- M-tile 1: SBUF-resident, runs back-to-back warm ----
            for k in range(NKT):
                for lo, hi in NSL:
                    nc.tensor.matmul(
                        ps_s[1][:, lo:hi],
                        tT[:, k * AT + 128:(k + 1) * AT],
                        vT[:, k * BV + lo:k * BV + hi],
                        start=(k == 0), stop=(k == NKT - 1))


            # ---- mid section in (i, half) streams: sp = tau*r_t*rv*S from
            # PSUM, E = exp(mask*sp), ES = sp*E, then grouped reduces ----
            sp = [bigp.tile([128, BV], BF16, name=f"sp{i}") for i in range(NMT)]
            big = [bigp.tile([128, 2 * BV], BF16, name=f"big{i}")
                   for i in range(NMT)]
            rhs_f = [smalls.tile([128, 128], BF16, name=f"rhs_f{i}")
                     for i in range(NMT)]
            red = [smalls.tile([128, 128], F32, name=f"red{i}")
                   for i in range(NMT)]
            for i in range(NMT):
                for lo, hi in HALF:
                    nc.vector.scalar_tensor_tensor(
                        sp[i][:, lo:hi], ps_s[i][:, lo:hi], tau_rt[i],
                        rv_bc[:, lo:hi], op0=MUL, op1=MUL)
                    nc.scalar.activation(big[i][:, BV + lo:BV + hi],
                                         sp[i][:, lo:hi], EXP,
                                         scale=maskt[:, i:i + 1])
            # ES multiplies first: they gate the v2t indicator matmuls (the
            # longest downstream chain); the reduces then fill DVE time while
            # PE/Scalar chew on the v2t->fe4 path
            for i in range(NMT):
                for lo, hi in HALF:
                    nc.vector.tensor_tensor(big[i][:, lo:hi], sp[i][:, lo:hi],
                                            big[i][:, BV + lo:BV + hi], op=MUL)
            # ---- PE keepalive while DVE/ACT chew the mid-section (ps_nv is
            # dead once the rsqrt chain has consumed it) ----
            for w in range(3):
                nc.tensor.matmul(ps_nv[:, 0:512], onesc, junk,
                                 start=True, stop=True)

            # ---- v2t: mask-folded indicator matmul over t; rhs is [ES|E] ----
            ps_v = psB.tile([8, 2 * BV], F32, tag="v")
            for i in range(NMT):
                for lo, hi in NSL3:
                    nc.tensor.matmul(ps_v[:, lo:hi], ind36m[i],
                                     big[i][:, lo:hi],
                                     start=(i == 0), stop=(i == NMT - 1))

            for i in range(NMT):
                # single grouped reduce over [ES | E]: columns 0:B are the
                # ES-sums, B:2B the E-sums; fills DVE time while PE runs the
                # ps_v matmuls
                nc.vector.reduce_sum(red[i],
                                     big[i].rearrange(
                                         "p (g v) -> p g v", v=V), axis=X)

            # ---- vps2 path at [36, x], half-split so DVE/ACT pipeline ----
            fe4 = bigp.tile([8, BV], BF16)
            d4 = smalls.tile([8, B], F32)
            v2ts = []
            for lo, hi in HALF:
                rdv = smalls.tile([8, 384], F32, name=f"rdv{lo}")
                nc.vector.reciprocal_approx_fast(rdv, ps_v[:8, BV + lo:BV + hi])
                v2t = smalls.tile([8, 384], F32, name=f"v2t{lo}")
                nc.vector.tensor_tensor(v2t, ps_v[:8, lo:hi], rdv, op=MUL)
                nc.scalar.activation(fe4[:, lo:hi], v2t, EXP)
                v2ts.append(v2t)



            # t2v/rhs: needed only by the final ps_o matmul, so it runs after
            # the fe4 chain on the DVE queue
            for i in range(NMT):
                rdn = smalls.tile([128, B], F32, name=f"rdn{i}")
                nc.vector.reciprocal_approx_fast(rdn, red[i][:, B:])
                t2v = smalls.tile([128, B], F32, name=f"t2v{i}")
                nc.vector.tensor_tensor(t2v, red[i][:, :B], rdn, op=MUL)
                nc.scalar.activation(rhs_f[i][:, B:], t2v, EXP)

            # ---- broadcast E4 over t-rows (PE), weight by sp, group-sum ----
            for i in range(NMT):
                ps_w = psA.tile([128, BV], F32, tag="s", name=f"ps_w{i}")
                for lo, hi in WSL:
                    nc.tensor.matmul(ps_w[:, lo:hi],
                                     indW[:, 128 * i:128 * (i + 1)],
                                     fe4[:, lo:hi], start=True, stop=True)
                w4s = sqp.tile([128, BV], BF16, name=f"w4s{i}")
                w4c = sqp.tile([128, BV], BF16, name=f"w4c{i}")
                hun = smalls.tile([128, B], F32, name=f"hun{i}")
                # split the ps_w*sp multiply across engines: half 0 direct
                # from PSUM on DVE, half 1 drained to bf16 by Scalar first
                # so its DVE multiply runs in 2x packed mode
                (l0, h0), (l1, h1) = HALF
                nc.scalar.copy(w4c[:, l1:h1], ps_w[:, l1:h1])
                nc.vector.tensor_tensor(w4s[:, l0:h0], ps_w[:, l0:h0],
                                        sp[i][:, l0:h0], op=MUL)
                nc.vector.tensor_tensor(w4s[:, l1:h1], w4c[:, l1:h1],
                                        sp[i][:, l1:h1], op=MUL)
                nc.vector.reduce_sum(hun,
                                     w4s.rearrange("p (g v) -> p g v", v=V),
                                     axis=X)
                nc.vector.tensor_tensor(rhs_f[i][:, :B], rhs_f[i][:, B:],
                                        hun, op=MUL)

            ps_o = psB.tile([8, 128], F32, tag="j")
            for i in range(NMT):
                nc.tensor.matmul(ps_o, ind36[:, 8 * i:8 * (i + 1)], rhs_f[i],
                                 start=(i == 0), stop=(i == NMT - 1))
            # d4 is only needed for the final denominator: reduce it late so
            # it does not sit on the DVE queue ahead of the hun reduces
            nc.vector.reduce_sum(d4,
                                 fe4.rearrange("p (g v) -> p g v", v=V),
                                 axis=X)
            d4t = smalls.tile([8, B], F32)
            nc.vector.tensor_scalar_mul(d4t, d4, TAU)
            dd = smalls.tile([8, B], F32)
            nc.vector.tensor_tensor(dd, ps_o[:8, B:], d4t, op=MUL)
            rdd = smalls.tile([8, B], F32)
            nc.vector.reciprocal_approx_fast(rdd, dd)
            outw = smalls.tile([8, B], F32)
            nc.vector.tensor_tensor(outw, ps_o[:8, :B], rdd, op=MUL)
            nc.sync.dma_start(out=out_d[:, :], in_=outw[:, :])

    nc.compile()
    return nc


_NC_CACHE = None


def _get_program():
    global _NC_CACHE
    if _NC_CACHE is None:
        _NC_CACHE = _build_program()
    return _NC_CACHE


def _make_in_maps(text_feat, video_feat, text_mask):
    # vT packed k-major, one DRAM tensor per k-chunk:
    # vT_k[p, c] = video[(b,v)=c, d=128k+p]
    vflat = video_feat.reshape(BV, D).astype(ml_dtypes.bfloat16)
    vT_ks = [np.ascontiguousarray(vflat.T[128 * k:128 * (k + 1), :])
             for k in range(NKT)]
    # ind36 slice i: column 4i + p//T is the block indicator; rows are
    # compact (4 per M-tile, 8 total) so every psum row is live.
    ind36 = np.zeros((128, NMT * 8), np.float32)
    for i in range(NMT):
        for p in range(128):
            ind36[p, 8 * i + 4 * i + p // T] = 1.0
    cpack = np.ones((128, NMT * 8 + 1), ml_dtypes.bfloat16)
    cpack[:, :NMT * 8] = ind36.astype(ml_dtypes.bfloat16)
    # indW slice i: [8, 128] with indW[r, p] = (r == 4i + p//T)
    indW = np.zeros((8, NMT * 128), ml_dtypes.bfloat16)
    for i in range(NMT):
        for p in range(128):
            indW[4 * i + p // T, 128 * i + p] = 1.0
    in_maps = []
    for c in range(NCORES):
        tsl = text_feat[c * AL:(c + 1) * AL].reshape(AT, D) \
            .astype(ml_dtypes.bfloat16)
        tT_b = np.ascontiguousarray(
            tsl.T.reshape(NKT, 128, AT).transpose(1, 0, 2)
            .reshape(128, NKT * AT))
        mask2 = np.ascontiguousarray(
            text_mask[c * AL:(c + 1) * AL].reshape(NMT, 128).T
            .astype(np.float32))
        im = {
            "tT": tT_b,
            "mask": mask2,
            "cpack": cpack,
            "indW": indW,
        }
        for k in range(NKT):
            im[f"vT{k}"] = vT_ks[k]
        in_maps.append(im)
    return in_maps


def kernel(text_feat, video_feat, text_mask, _trace=False):
    text_feat = np.asarray(text_feat, dtype=np.float32)
    video_feat = np.asarray(video_feat, dtype=np.float32)
    text_mask = np.asarray(text_mask)
    nc = _get_program()
    in_maps = _make_in_maps(text_feat, video_feat, text_mask)
    res = run_bass_kernel_spmd(nc, in_maps, core_ids=list(range(NCORES)),
                               trace=_trace)
    out = np.concatenate([res.results[c]["out"] for c in range(NCORES)], axis=0)
    if _trace:
        kernel.last_exec_time_ns = res.exec_time_ns
        kernel.last_results = res
    return out

